# revision 45
# baseline (speedup 1.0000x reference)
"""MHA kernel for trn2: 8 cores = 2 (batch DP) x 4 (head TP, 4 heads/core).

bf16 matmul edition (PE runs bf16 at 1 cycle/row vs 2 for fp32 HIGH):
  - x^T, Wq/Wk/Wv/Wo, cos/sin, perm, tri all staged bf16; PSUM stays f32
  - Q^T/K^T computed as [d, t] via lhsT=W-slice, rhs=x^T; RoPE via PE
    half-swap permutation matmul + signed sin table, all bf16
  - V computed as [t, d] bf16 (plus ones column for softmax denominators)
  - S^T duos [tk=128, 2*512]; 2-head packing on the PE (K=64, base
    partitions 0/64); causal block skipping; exp per duo (ACT, bf16 out);
    software-pipelined AV one duo behind S
  - AV accumulates O_aug^T [65, tq] f32 PSUM per head; row 64 = denom
  - normalize: broadcast d via tiny f32r matmul, reciprocal_approx_fast
    on DVE (no ACT table thrash), multiply -> y chunk bf16
  - AllGather y^T per (512-col chunk, head-pair) in bf16 (8 CC ops),
    overlapped with the next attention section; projection chunks run
    one AG behind; proj bias folded in via a ones-row matmul so the
    eviction is a plain DVE copy (ACT stays exp-only in phase 2)
Host reassembles: concat cout slices, transpose, stack batches.
"""

import sys

sys.path.insert(0, "/opt/trn_rl_repo")

from contextlib import ExitStack  # noqa: E402

import numpy as np  # noqa: E402
import ml_dtypes  # noqa: E402

import concourse.bacc as bacc  # noqa: E402
import concourse.bass as bass  # noqa: E402
import concourse.tile as tile  # noqa: E402
from concourse import mybir  # noqa: E402
from concourse.bass_utils import run_bass_kernel_spmd  # noqa: E402

B, T, C, H = 2, 2048, 1024, 16
HD, HD2 = 64, 32
NCORES, GROUPS, HPG, NPAIRS = 8, 4, 4, 2
TK, TQ = 128, 512
NQ = T // TQ  # 4 q-chunks
NKT = T // TK  # 16 tk tiles
KT = C // 128  # 8 contraction tiles
DGRP = 256  # head dims per core (4 heads * 64)
NIDX = NPAIRS * NQ  # 8 (p, qi) output blocks

F32 = mybir.dt.float32
F32R = mybir.dt.float32r
BF16 = mybir.dt.bfloat16
AF = mybir.ActivationFunctionType
ALU = mybir.AluOpType
SCALE = 1.0 / 8.0  # 1/sqrt(HD)
BF = ml_dtypes.bfloat16


def r32(ap):
    return ap.bitcast(F32R)


def build_nc():
    nc = bacc.Bacc(target_bir_lowering=False)

    xr = nc.dram_tensor("xr", [128, KT * T], BF16, kind="ExternalInput")
    wqr = nc.dram_tensor("wqr", [128, KT * DGRP], BF16, kind="ExternalInput")
    wkr = nc.dram_tensor("wkr", [128, KT * DGRP], BF16, kind="ExternalInput")
    wvr = nc.dram_tensor("wvr", [128, KT * DGRP], BF16, kind="ExternalInput")
    wor = nc.dram_tensor("wor", [128, KT * DGRP], BF16, kind="ExternalInput")
    cos4 = nc.dram_tensor("cos4", [128, T], BF16, kind="ExternalInput")
    sin4 = nc.dram_tensor("sin4", [128, T], BF16, kind="ExternalInput")
    perm = nc.dram_tensor("perm", [128, 128], BF16, kind="ExternalInput")
    tri = nc.dram_tensor("tri", [TK, TK], BF16, kind="ExternalInput")
    bq = nc.dram_tensor("bq", [DGRP, 1], F32, kind="ExternalInput")
    bk = nc.dram_tensor("bk", [DGRP, 1], F32, kind="ExternalInput")
    wob = nc.dram_tensor("wob", [1, DGRP], BF16, kind="ExternalInput")
    sel2 = nc.dram_tensor("sel2", [1, 2 * 128], F32, kind="ExternalInput")
    out = nc.dram_tensor("out", [DGRP, T], F32, kind="ExternalOutput")

    with tile.TileContext(nc) as tc, ExitStack() as top:
        dram = top.enter_context(tc.tile_pool(name="dram", bufs=1, space="DRAM"))
        # qi 0..2: one AG per q-chunk; qi 3 split per pair so the tail
        # projection can start before the final AG lands
        y_loc = [dram.tile([DGRP, TQ], BF16, name=f"ylq{q}") for q in range(NQ - 1)]
        y_all = [
            dram.tile([GROUPS * DGRP, TQ], BF16, name=f"yaq{q}") for q in range(NQ - 1)
        ]
        y_loc3 = [dram.tile([128, TQ], BF16, name=f"yl3p{p}") for p in range(NPAIRS)]
        y_all3 = [
            dram.tile([GROUPS * 128, TQ], BF16, name=f"ya3p{p}") for p in range(NPAIRS)
        ]
        warm_in = dram.tile([128, TQ], BF16, name="warm_in")
        warm_out = dram.tile([GROUPS * 128, TQ], BF16, name="warm_out")
        consts = top.enter_context(tc.tile_pool(name="consts", bufs=1))
        cos_sb = consts.tile([128, T], BF16)
        sin_sb = consts.tile([128, T], BF16)
        perm_sb = consts.tile([128, 128], BF16)
        tri_sb = consts.tile([TK, TK], BF16)
        bq_sb = consts.tile([128, NPAIRS], F32)
        bk_sb = consts.tile([128, NPAIRS], F32)
        wob_sb = consts.tile([1, DGRP], BF16)
        ones_sb = consts.tile([1, TQ], BF16)
        sel2_sb = consts.tile([1, 2, 128], F32)
        bqr = bq.ap().rearrange("(p c) one -> c (p one)", c=128)
        bkr = bk.ap().rearrange("(p c) one -> c (p one)", c=128)

        wo_pool = top.enter_context(tc.tile_pool(name="wo", bufs=1))
        wo_sb = wo_pool.tile([128, KT, DGRP], BF16)

        rqk_pool = top.enter_context(tc.tile_pool(name="rqk", bufs=1))
        # RQ/RK per pair: [128, T]; rows = (u1 h0, u2 h0, u1 h1, u2 h1) x 32
        RQ = [rqk_pool.tile([128, T], BF16, name=f"RQ{p}") for p in range(NPAIRS)]
        RK = [rqk_pool.tile([128, T], BF16, name=f"RK{p}") for p in range(NPAIRS)]
        v_pool = top.enter_context(tc.tile_pool(name="vsb", bufs=1))
        V_sb = v_pool.tile([128, NKT, HPG, HD + 1], BF16)

        nc.vector.memset(ones_sb, 1.0)

        # ---------------- Phase 1: QKV projection + RoPE ----------------
        with ExitStack() as ph1:
            xt_pool = ph1.enter_context(tc.tile_pool(name="xt", bufs=1))
            xT_sb = xt_pool.tile([128, NQ, KT, TQ], BF16)
            wqk_pool = ph1.enter_context(tc.tile_pool(name="wqk", bufs=1))
            wq_sb = wqk_pool.tile([128, KT, DGRP], BF16)
            wk_sb = wqk_pool.tile([128, KT, DGRP], BF16)
            wv_sb = wqk_pool.tile([128, KT, DGRP], BF16)

            # DMA issue order = completion order per ring. x blocks go on
            # the sync HWDGE ring, weights/consts on the gpsimd SWDGE ring,
            # so the first Q matmul's two inputs load in parallel.
            # warm AG first on the gpsimd stream: absorbs the slow first
            # CC op during phase 1 while the CC stream is otherwise idle
            nc.gpsimd.collective_compute(
                "AllGather",
                ALU.bypass,
                ins=[warm_in.opt()],
                outs=[warm_out.opt()],
                replica_groups=[[0, 1, 2, 3], [4, 5, 6, 7]],
            )
            xrv = xr.ap().rearrange("p (n k t) -> p n k t", n=NQ, k=KT)
            wqv = wqr.ap().rearrange("p (k d) -> p k d", k=KT)
            # first k-tile of x chunk 0 + first wq slice land first so the
            # opening Q matmul starts ~8us earlier
            nc.sync.dma_start(out=xT_sb[:, 0, 0], in_=xrv[:, 0, 0])
            nc.gpsimd.dma_start(out=wq_sb[:, 0], in_=wqv[:, 0])
            nc.sync.dma_start(out=xT_sb[:, 0, 1:], in_=xrv[:, 0, 1:])
            nc.gpsimd.dma_start(out=wq_sb[:, 1:], in_=wqv[:, 1:])
            nc.scalar.dma_start(out=xT_sb[:, 1], in_=xrv[:, 1])
            nc.sync.dma_start(out=xT_sb[:, 2], in_=xrv[:, 2])
            nc.scalar.dma_start(out=xT_sb[:, 3], in_=xrv[:, 3])
            nc.gpsimd.dma_start(out=perm_sb, in_=perm.ap())
            nc.gpsimd.dma_start(out=bq_sb, in_=bqr)
            nc.gpsimd.dma_start(out=bk_sb, in_=bkr)
            nc.gpsimd.dma_start(out=wk_sb, in_=wkr.ap())
            nc.gpsimd.dma_start(out=cos_sb, in_=cos4.ap())
            nc.gpsimd.dma_start(out=sin_sb, in_=sin4.ap())
            nc.gpsimd.dma_start(out=wv_sb, in_=wvr.ap())
            nc.gpsimd.dma_start(out=tri_sb, in_=tri.ap())
            nc.gpsimd.dma_start(out=wob_sb, in_=wob.ap())
            nc.gpsimd.dma_start(out=r32(sel2_sb), in_=sel2.ap())
            nc.gpsimd.dma_start(out=wo_sb, in_=wor.ap())

            ps1 = ph1.enter_context(tc.tile_pool(name="ps1", bufs=2, space="PSUM"))
            tmp_pool = ph1.enter_context(tc.tile_pool(name="tmp", bufs=4))

            def rope_tail(Rc, n):
                # swap 32-row halves via PE perm matmul; sign baked in sin_sb
                sw_ps = ps1.tile([128, TQ], F32, name="sw_ps")
                nc.tensor.matmul(
                    out=sw_ps, lhsT=perm_sb, rhs=Rc, start=True, stop=True
                )
                tmpS = tmp_pool.tile([128, TQ], BF16, name="tmpS")
                tmpC = tmp_pool.tile([128, TQ], BF16, name="tmpC")
                nc.vector.tensor_mul(tmpS, sw_ps, sin_sb[:, n * TQ : (n + 1) * TQ])
                nc.vector.tensor_mul(tmpC, Rc, cos_sb[:, n * TQ : (n + 1) * TQ])
                nc.vector.tensor_add(Rc, tmpC, tmpS)

            pend = None
            for p in range(NPAIRS):
                for w_sb, b_sb, R in (
                    (wq_sb, bq_sb, RQ[p]),
                    (wk_sb, bk_sb, RK[p]),
                ):
                    for n in range(NQ):
                        u_ps = ps1.tile([128, TQ], F32, name="u_ps")
                        for k in range(KT):
                            nc.tensor.matmul(
                                out=u_ps,
                                lhsT=w_sb[:, k, p * 128 : (p + 1) * 128],
                                rhs=xT_sb[:, n, k, :],
                                start=(k == 0),
                                stop=(k == KT - 1),
                            )
                        Rc = R[:, n * TQ : (n + 1) * TQ]
                        # evict with bias -> R buffer (pre-rotation values)
                        nc.scalar.activation(
                            out=Rc,
                            in_=u_ps,
                            func=AF.Identity,
                            bias=b_sb[:, p : p + 1],
                        )
                        if pend is not None:
                            rope_tail(*pend)
                        pend = (Rc, n)
            rope_tail(*pend)

            # V tiles [t,d] with ones column per head
            nc.vector.memset(V_sb, 1.0)
            for tt in range(NKT):
                v_ps = ps1.tile([128, DGRP], F32, name="v_ps")
                for k in range(KT):
                    nc.tensor.matmul(
                        out=v_ps,
                        lhsT=xT_sb[:, tt // 4, k, (tt % 4) * TK : (tt % 4 + 1) * TK],
                        rhs=wv_sb[:, k, :],
                        start=(k == 0),
                        stop=(k == KT - 1),
                    )
                nc.vector.tensor_copy(
                    out=V_sb[:, tt, :, 0:HD],
                    in_=v_ps.rearrange("p (h d) -> p h d", h=HPG),
                )

        # ---------- Phase 2+3: attention, chunked AG + projection ----------
        with ExitStack() as ph2:
            sd_pool = ph2.enter_context(tc.tile_pool(name="sduo", bufs=2, space="PSUM"))
            av_pool = ph2.enter_context(tc.tile_pool(name="av", bufs=1, space="PSUM"))
            # bc_ps and o_ps share one double-buffered pool (2 banks) so a
            # proj eviction never blocks the next accumulation chain
            ps3 = ph2.enter_context(tc.tile_pool(name="ps3", bufs=2, space="PSUM"))
            bc_pool = ps3
            pt_pool = ph2.enter_context(tc.tile_pool(name="ptile", bufs=3))
            o_pool = ph2.enter_context(tc.tile_pool(name="osb", bufs=3))
            ya_pool = ph2.enter_context(tc.tile_pool(name="ya", bufs=2))
            ob_pool = ph2.enter_context(tc.tile_pool(name="ob", bufs=2))
            d_pool = ph2.enter_context(tc.tile_pool(name="dsb", bufs=1))
            dq_pool = ph2.enter_context(tc.tile_pool(name="dq", bufs=2))
            r2_pool = ph2.enter_context(tc.tile_pool(name="rb2", bufs=2))
            o_all = d_pool.tile([128, NIDX, TQ], F32)
            av = [av_pool.tile([128, TQ], F32, name=f"av{hh}") for hh in range(2)]

            def issue_av(p, qi, ptile, g2, hh):
                last_ti = 4 * qi + 3
                for ji in range(2):
                    ti = 2 * g2 + ji
                    nc.tensor.matmul(
                        out=av[hh][0:65, :],
                        lhsT=V_sb[:, ti, 2 * p + hh, :],
                        rhs=ptile[:, ji * TQ : (ji + 1) * TQ],
                        start=(ti == 0),
                        stop=(ti == last_ti),
                    )

            def proj_chunk(qi):
                # k-tile k = (group k//2, pair k%2). qi<3: one gather DMA.
                # qi==3: per-pair DMAs, p0 k-tiles accumulated first so the
                # tail chunk starts before its p1 AG lands.
                ya_sb = ya_pool.tile([128, KT, TQ], BF16, name="ya_sb")
                if qi < NQ - 1:
                    nc.sync.dma_start(
                        out=ya_sb,
                        in_=y_all[qi].rearrange("(k c) t -> c k t", c=128),
                    )
                    korder = list(range(KT))
                else:
                    for p in range(NPAIRS):
                        nc.sync.dma_start(
                            out=ya_sb.rearrange("c (g p) t -> c p g t", p=2)[:, p],
                            in_=y_all3[p].rearrange("(g c) t -> c g t", c=128),
                        )
                    korder = [0, 2, 4, 6, 1, 3, 5, 7]
                for mb in range(2):
                    o_ps = ps3.tile([128, TQ], F32, name="ps_sc")
                    for i, k in enumerate(korder):
                        nc.tensor.matmul(
                            out=o_ps,
                            lhsT=wo_sb[:, k, mb * 128 : (mb + 1) * 128],
                            rhs=ya_sb[:, k, :],
                            start=(i == 0),
                            stop=False,
                        )
                    # bias via ones-row matmul: keeps the eviction off ACT
                    nc.tensor.matmul(
                        out=o_ps,
                        lhsT=wob_sb[0:1, mb * 128 : (mb + 1) * 128],
                        rhs=ones_sb,
                        start=False,
                        stop=True,
                    )
                    ob = ob_pool.tile([128, TQ], F32, name="ob")
                    nc.vector.tensor_copy(out=ob, in_=o_ps)
                    nc.sync.dma_start(
                        out=out.ap()[
                            mb * 128 : (mb + 1) * 128, qi * TQ : (qi + 1) * TQ
                        ],
                        in_=ob,
                    )

            def emit_norm(qi, p):
                # denominators first -- they head the AG critical path.
                # Stage in free dim of partition 0 (engine partition
                # bases must be 32-aligned), broadcast via two K=1
                # matmuls reading dq directly (no scatter DMA needed).
                idx = qi * NPAIRS + p
                dq = dq_pool.tile([1, 2, TQ], F32, name="dq")
                nc.vector.tensor_copy(out=r32(dq[0:1, 0, :]), in_=av[0][64:65, :])
                nc.vector.tensor_copy(out=r32(dq[0:1, 1, :]), in_=av[1][64:65, :])
                bc_ps = bc_pool.tile([128, TQ], F32, name="ps_sc")
                for j in range(2):
                    nc.tensor.matmul(
                        out=bc_ps,
                        lhsT=r32(sel2_sb[0:1, j, :]),
                        rhs=r32(dq[0:1, j, :]),
                        start=(j == 0),
                        stop=(j == 1),
                    )
                nc.vector.tensor_copy(out=o_all[0:64, idx, :], in_=av[0][0:64, :])
                nc.vector.tensor_copy(out=o_all[64:128, idx, :], in_=av[1][0:64, :])
                rb2 = r2_pool.tile([128, TQ], F32, name="rb2")
                nc.vector.reciprocal_approx_fast(out=rb2, in_=bc_ps)
                yt2 = o_pool.tile([128, TQ], BF16, name="yt2")
                nc.vector.tensor_mul(yt2, o_all[:, idx, :], rb2)
                if qi < NQ - 1:
                    nc.gpsimd.dma_start(
                        out=y_loc[qi][p * 128 : (p + 1) * 128, :], in_=yt2
                    )
                    if p == 1:
                        nc.gpsimd.collective_compute(
                            "AllGather",
                            ALU.bypass,
                            ins=[y_loc[qi].opt()],
                            outs=[y_all[qi].opt()],
                            replica_groups=[[0, 1, 2, 3], [4, 5, 6, 7]],
                        )
                else:
                    nc.gpsimd.dma_start(out=y_loc3[p], in_=yt2)
                    nc.gpsimd.collective_compute(
                        "AllGather",
                        ALU.bypass,
                        ins=[y_loc3[p].opt()],
                        outs=[y_all3[p].opt()],
                        replica_groups=[[0, 1, 2, 3], [4, 5, 6, 7]],
                    )
                # proj chunks run 2+ sections after their AG trigger so
                # cross-core start skew can't stall them; the last two stay
                # at the tail to cover the final AG's flight
                if (qi, p) == (2, 1):
                    proj_chunk(0)
                elif (qi, p) == (3, 0):
                    proj_chunk(1)
                elif (qi, p) == (3, 1):
                    proj_chunk(2)
                    proj_chunk(3)

            pend_norm = None
            for qi in range(NQ):
                for p in range(NPAIRS):
                    avpend = None
                    for g2 in range(2 * qi + 2):
                        for hh in range(2):
                            s_ps = sd_pool.tile([128, 2 * TQ], F32, name="s_ps")
                            for ji in range(2):
                                ti = 2 * g2 + ji
                                nc.tensor.matmul(
                                    out=s_ps[:, ji * TQ : (ji + 1) * TQ],
                                    lhsT=RK[p][
                                        64 * hh : 64 * hh + 64,
                                        ti * TK : (ti + 1) * TK,
                                    ],
                                    rhs=RQ[p][
                                        64 * hh : 64 * hh + 64,
                                        qi * TQ : (qi + 1) * TQ,
                                    ],
                                    start=True,
                                    stop=True,
                                )
                            # previous section's normalize goes here: its
                            # DVE chain runs under this section's opening
                            # S matmuls instead of stalling the PE
                            if pend_norm is not None:
                                emit_norm(*pend_norm)
                                pend_norm = None
                            ptile = pt_pool.tile([128, 2 * TQ], BF16, name="ptile")
                            nc.scalar.activation(
                                out=ptile, in_=s_ps, func=AF.Exp, scale=SCALE
                            )
                            if g2 >= 2 * qi:  # diagonal duo: causal masking
                                for ji in range(2):
                                    ti = 2 * g2 + ji
                                    off = TK * (ti - 4 * qi)
                                    col = ji * TQ
                                    if off > 0:
                                        nc.vector.memset(
                                            ptile[:, col : col + off], 0.0
                                        )
                                    blk = ptile[:, col + off : col + off + TK]
                                    nc.vector.tensor_mul(blk, blk, tri_sb)
                            if avpend is not None:
                                issue_av(p, qi, *avpend)
                            avpend = (ptile, g2, hh)
                    issue_av(p, qi, *avpend)
                    pend_norm = (qi, p)
            emit_norm(*pend_norm)
    nc.finalize()
    return nc


_NC = None


def _get_nc():
    global _NC
    if _NC is None:
        _NC = build_nc()
    return _NC


def _relay(w):
    # [KT*128, M] -> [128, KT*M] so each partition's DMA line is contiguous
    kt, m = w.shape[0] // 128, w.shape[1]
    return np.ascontiguousarray(
        w.reshape(kt, 128, m).transpose(1, 0, 2).reshape(128, kt * m).astype(BF)
    )


def _relay_x(xb):
    # x^T [C, T] -> [128, NQ*KT*TQ] n-major so each 512-col block is one
    # contiguous-per-partition DMA
    xt = xb.T.reshape(KT, 128, NQ, TQ)
    return np.ascontiguousarray(
        xt.transpose(1, 2, 0, 3).reshape(128, NQ * KT * TQ).astype(BF)
    )


def _in_maps(x, freqs_cos, freqs_sin, Wqkv, bqkv, Wproj, bproj):
    x = np.asarray(x, np.float32)
    Wqkv = np.asarray(Wqkv, np.float32)
    bqkv = np.asarray(bqkv, np.float32)
    Wproj = np.asarray(Wproj, np.float32)
    bproj = np.asarray(bproj, np.float32)
    cos4 = np.ascontiguousarray(
        np.tile(np.asarray(freqs_cos, np.float32).T, (4, 1)).astype(BF)
    )
    sinT = np.asarray(freqs_sin, np.float32).T  # [32, T]
    sin4 = np.ascontiguousarray(
        np.tile(np.concatenate([-sinT, sinT], axis=0), (2, 1)).astype(BF)
    )
    perm = np.zeros((128, 128), np.float32)
    for j in range(128):
        i = j + 32 if (j % 64) < 32 else j - 32
        perm[i, j] = 1.0
    perm = perm.astype(BF)
    tri = np.triu(np.ones((TK, TK), np.float32)).astype(BF)
    # slot 0 selects head-even denom into partitions 0:64, slot 1 head-odd
    sel2 = np.zeros((1, 2, 128), np.float32)
    sel2[0, 0, 0:64] = 1.0
    sel2[0, 1, 64:128] = 1.0
    sel2 = sel2.reshape(1, 256)
    bproj_eff = bproj + bqkv[2 * C : 3 * C] @ Wproj
    maps = []
    for r in range(NCORES):
        b, g = r // GROUPS, r % GROUPS
        sl = slice(DGRP * g, DGRP * (g + 1))
        maps.append(
            {
                "xr": _relay_x(x[b]),
                "wqr": _relay(Wqkv[:, 0 * C :][:, sl]),
                "wkr": _relay(Wqkv[:, 1 * C :][:, sl]),
                "wvr": _relay(Wqkv[:, 2 * C :][:, sl]),
                "wor": _relay(Wproj[:, sl]),
                "cos4": cos4,
                "sin4": sin4,
                "perm": perm,
                "tri": tri,
                "bq": np.ascontiguousarray(bqkv[0 * C : 1 * C][sl]).reshape(DGRP, 1),
                "bk": np.ascontiguousarray(bqkv[1 * C : 2 * C][sl]).reshape(DGRP, 1),
                "wob": np.ascontiguousarray(bproj_eff[sl]).reshape(1, DGRP).astype(BF),
                "sel2": sel2,
            }
        )
    return maps


def _assemble(results):
    y = np.empty((B, T, C), np.float32)
    for b in range(B):
        cat = np.concatenate(
            [np.asarray(results[GROUPS * b + g]["out"]) for g in range(GROUPS)], axis=0
        )
        y[b] = cat.T
    return y


def kernel(**inputs):
    nc = _get_nc()
    res = run_bass_kernel_spmd(nc, _in_maps(**inputs), core_ids=list(range(NCORES)))
    return _assemble(res.results)


def kernel_traced(**inputs):
    import tempfile

    nc = _get_nc()
    tmpdir = tempfile.mkdtemp(prefix="mha_trace_")
    res = run_bass_kernel_spmd(
        nc,
        _in_maps(**inputs),
        core_ids=list(range(NCORES)),
        trace=True,
        trace_cores=list(range(NCORES)),
        tmpdir=tmpdir,
    )
    return _assemble(res.results), res.exec_time_ns, tmpdir
